# revision 18
# baseline (speedup 1.0000x reference)
"""ACmix (local 3x3 window attention + dynamic conv mix) on 8 TRN2 NeuronCores.

Sharding: data-parallel over batch B=8, one batch element per core.
Per-core layout: channels (128) on partitions, L = H*W = 4096 on the free dim.

Math per core (x: [128, L]):
  qkv   = w_qkv @ x                       (PE, 3 matmuls)
  q,k,v = split(qkv)
  k_pad, v_pad: zero-padded [128, 66*66] spatial buffers; a 3x3 shift is an AP offset
  logits[9h+s, l] = sum_{d in head h} q[d,l] * k_pad[d, l+off_s]
      -> DVE products q * k_shift (bf16, 2x), PE block-ones reduce into PSUM
  e = exp(logits); sums replicated per row via [36,36] block-ones matmul;
  r_exp = exp(-ln(sums)+ln|alpha|)  ([36, L], no round-trip needed)
  c[9h+s, l] = (beta*kernel + beta*b) +/- e*r_exp   (kernel = blockdiag(w_kernel)@qkv)
  c broadcast to 128 partitions via DRAM round-trip replication DMA
  out = w_proj @ sum_s c_exp_s * v_pad[:, l+off_s]   (PE accumulation, proj folded in)
"""

import os
from contextlib import ExitStack

import numpy as np
import ml_dtypes

import concourse.bass as bass
import concourse.bacc as bacc
import concourse.tile as tile
from concourse import mybir
from concourse.bass_utils import run_bass_kernel_spmd

BF16 = mybir.dt.bfloat16
F32 = mybir.dt.float32
NPBF16 = ml_dtypes.bfloat16

B, C, H, W = 8, 128, 64, 64
L = H * W                      # 4096
NH, HD, K2 = 4, 32, 9
PW, PH = W + 2, H + 2          # 66
PL = PW * PH                   # 4356
SHIFTS = [(di, dj) for di in (-1, 0, 1) for dj in (-1, 0, 1)]  # k2 = 3(di+1)+(dj+1)
NCHUNK = L // 512              # 8
NQ = L // 1024                 # 4 quarters for stage-1 products

# weight blob column layout (bf16, 128 partitions)
OFF_QKV = 0            # wqkvT [128, 384]
OFF_PROJ = 384         # wprojT [128, 128]
OFF_WKB = 512          # wkbT 3 x [128, 36]
OFF_S2 = 620           # sones2 [36, 36] (rows 36.. zero)
OFF_BONES = 656        # bones 9 x [128, 36]
OFF_EXP = 656 + 9 * 36  # 980: E_s [36 rows, 128] x PE-shifts
PE_SHIFTS = [3, 5, 8]
DMA_SHIFTS = [0, 1, 2, 4, 6, 7]
S2_ORDER = [0, 3, 1, 5, 2, 8, 4, 6, 7]
WBLOB = 980 + len(PE_SHIFTS) * 128

TRACE = False
LAST_RESULTS = None


def _ensure_profile_hook():
    """Provide antenv.axon_hooks (missing in this container's antenv stub)
    so run_bass_kernel_spmd(trace=True) can capture NTFF profiles."""
    import sys, types
    try:
        from antenv.axon_hooks import get_axon_ntff_profile_hook  # noqa: F401
        return
    except ImportError:
        pass
    try:
        from trn_agent_boot.trn_boot import _ntff_profile_via_ctypes
        hook = _ntff_profile_via_ctypes("/opt/axon/libaxon_pjrt.so")
    except Exception:
        hook = None
    mod = types.ModuleType("antenv.axon_hooks")
    state = {"hook": hook}
    mod.get_axon_ntff_profile_hook = lambda: state["hook"]
    mod.set_axon_ntff_profile_hook = lambda h: state.__setitem__("hook", h)
    sys.modules["antenv.axon_hooks"] = mod
    import antenv
    antenv.axon_hooks = mod


def _build(subtract_m: bool):
    nc = bacc.Bacc("TRN2", target_bir_lowering=False, debug=False)
    x_ext = nc.declare_dram_parameter("x", [C, L], BF16, isOutput=False)
    wblob_ext = nc.declare_dram_parameter("wblob", [C, WBLOB], BF16, isOutput=False)
    bias_ext = nc.declare_dram_parameter("bias", [36, 1], F32, isOutput=False)
    lnalpha_ext = nc.declare_dram_parameter("lnalpha", [36, 1], F32, isOutput=False)
    out_ext = nc.declare_dram_parameter("out", [C, L], BF16, isOutput=True)

    with tile.TileContext(nc) as tc, ExitStack() as ctx:
        pw = ctx.enter_context(tc.tile_pool(name="weights", bufs=1))
        pmain = ctx.enter_context(tc.tile_pool(name="main", bufs=1))
        pdram = ctx.enter_context(tc.tile_pool(name="dram", bufs=1, space="DRAM"))
        c_dram = pdram.tile([36, L], BF16, name="c_rt")

        # ---- S0: weights + input DMA, pad-border memsets -------------------
        wblob = pw.tile([C, WBLOB], BF16)
        nc.sync.dma_start(wblob[:], wblob_ext[:])
        bias_sb = pw.tile([36, 1], F32)
        nc.sync.dma_start(bias_sb[:], bias_ext[:])
        lnalpha = pw.tile([36, 1], F32)
        nc.sync.dma_start(lnalpha[:], lnalpha_ext[:])

        wqkvT = wblob[:, OFF_QKV : OFF_QKV + 384]
        wprojT = wblob[:, OFF_PROJ : OFF_PROJ + 128]
        wkbT = [wblob[:, OFF_WKB + 36 * j : OFF_WKB + 36 * j + 36] for j in range(3)]
        sones2 = wblob[0:36, OFF_S2 : OFF_S2 + 36]
        bones = [wblob[:, OFF_BONES + 36 * s : OFF_BONES + 36 * s + 36] for s in range(9)]

        k_pad = pmain.tile([C, PL], BF16)
        v_pad = pmain.tile([C, PL], BF16)
        k_stag = pmain.tile([C, PL], BF16)
        v_stag = pmain.tile([C, PL], BF16)
        for t in (k_pad, v_pad):
            t3 = t[:].rearrange("p (r c) -> p r c", c=PW)
            nc.gpsimd.memset(t3[:, 0, :], 0.0)          # top padded row
            nc.gpsimd.memset(t3[:, PH - 1, :], 0.0)     # bottom padded row
            nc.gpsimd.memset(t3[:, 1 : PH - 1, 0:1], 0.0)
            nc.gpsimd.memset(t3[:, 1 : PH - 1, PW - 1 : PW], 0.0)

        with tc.tile_pool(name="early", bufs=1) as pearly, \
             tc.tile_pool(name="prods", bufs=2) as pprod:
            x_sb = pearly.tile([C, L], BF16)
            for xq in range(4):
                nc.sync.dma_start(x_sb[:, xq * 1024 : xq * 1024 + 1024],
                                  x_ext[:, xq * 1024 : xq * 1024 + 1024])
            q_sb = pearly.tile([C, L], BF16)
            kern_sb = pmain.tile([36, L], BF16)
            e_sb = pmain.tile([36, L], BF16)
            t_sb = pmain.tile([36, L], F32)
            r_exp = pmain.tile([36, L], BF16)
            c_sb = pmain.tile([36, L], BF16)
            c_ap = c_dram[:]

            kp3 = k_pad[:].rearrange("p (r c) -> p r c", c=PW)
            vp3 = v_pad[:].rearrange("p (r c) -> p r c", c=PW)
            ks3 = k_stag[:].rearrange("p (r c) -> p r c", c=PW)
            vs3 = v_stag[:].rearrange("p (r c) -> p r c", c=PW)

            # ---- Phase A: qkv = w_qkv @ x (k first so products start early)
            with tc.tile_pool(name="psQ", bufs=2, space="PSUM") as psQ:
                for t in (1, 0, 2):  # k, q, v
                    for hf in range(2):
                        ps = psQ.tile([C, 2048], F32, tag="qkv")
                        for mc in range(4):
                            col = hf * 2048 + mc * 512
                            nc.tensor.matmul(
                                ps[:, mc * 512 : mc * 512 + 512],
                                wqkvT[:, t * C : t * C + C],
                                x_sb[:, col : col + 512],
                                start=True, stop=True,
                            )
                        if t == 0:
                            nc.scalar.copy(q_sb[:, hf * 2048 : hf * 2048 + 2048], ps[:])
                        else:
                            dst = (k_pad if t == 1 else v_pad)[:].rearrange(
                                "p (r c) -> p r c", c=PW
                            )
                            r0 = hf * 32
                            nc.scalar.copy(
                                dst[:, 1 + r0 : 1 + r0 + 32, 1 : 1 + W],
                                ps[:].rearrange("p (r c) -> p r c", c=W),
                            )
                    if t == 1:
                        nc.gpsimd.tensor_copy(k_stag[:, 0 : PL - 2], k_pad[:, 1 : PL - 1])
                    elif t == 2:
                        nc.gpsimd.tensor_copy(v_stag[:, 0 : PL - 2], v_pad[:, 1 : PL - 1])

            # ---- Phase B: fused per-half pipeline --------------------------
            eexp = {s: wblob[0:36, OFF_EXP + 128 * i : OFF_EXP + 128 * i + 128]
                    for i, s in enumerate(PE_SHIFTS)}
            with tc.tile_pool(name="psK", bufs=1, space="PSUM") as psK, \
                 tc.tile_pool(name="psB", bufs=2, space="PSUM") as psB:
                for hf in range(2):
                    sl = slice(hf * 2048, hf * 2048 + 2048)
                    # kernel proj for this half
                    psk = psK.tile([36, 2048], F32, tag="kern", name=f"kern{hf}")
                    r0 = hf * 32
                    rhs_k = kp3[:, 1 + r0 : 1 + r0 + 32, 1 : 1 + W]
                    rhs_v = vp3[:, 1 + r0 : 1 + r0 + 32, 1 : 1 + W]
                    for mc in range(4):
                        col = hf * 2048 + mc * 512
                        rr = mc * 8
                        nc.tensor.matmul(psk[:, mc * 512 : mc * 512 + 512], wkbT[0],
                                         q_sb[:, col : col + 512], start=True, stop=False)
                        nc.tensor.matmul(psk[:, mc * 512 : mc * 512 + 512], wkbT[1],
                                         rhs_k[:, rr : rr + 8, :], start=False, stop=False)
                        nc.tensor.matmul(psk[:, mc * 512 : mc * 512 + 512], wkbT[2],
                                         rhs_v[:, rr : rr + 8, :], start=False, stop=True)
                    nc.scalar.activation(
                        kern_sb[:, sl], psk[:],
                        mybir.ActivationFunctionType.Identity, bias=bias_sb[:],
                    )

                    for qq in range(2):
                        qt = hf * 2 + qq
                        prods = []
                        # dj != 0 shifts first: they don't wait on the stag copy
                        order = [s for s in range(9) if SHIFTS[s][1] != 0] + \
                                [s for s in range(9) if SHIFTS[s][1] == 0]
                        prodmap = {}
                        for s in order:
                            di, dj = SHIFTS[s]
                            pr = pprod.tile([C, 1024], BF16, tag=f"pr{s}", name=f"pr{s}_{qt}")
                            r0q = qt * 16
                            if dj == 0:
                                psrc = ks3[:, 1 + di + r0q : 1 + di + r0q + 16, 0:W]
                            else:
                                psrc = kp3[:, 1 + di + r0q : 1 + di + r0q + 16,
                                           1 + dj : 1 + dj + W]
                            nc.vector.tensor_mul(
                                pr[:].rearrange("p (r c) -> p r c", c=W),
                                q_sb[:, qt * 1024 : qt * 1024 + 1024].rearrange(
                                    "p (r c) -> p r c", c=W),
                                psrc,
                            )
                            prodmap[s] = pr
                        for sub in range(2):
                            ci = qt * 2 + sub
                            lg = psB.tile([36, 512], F32, tag="logits", name=f"lg{ci}")
                            for s in range(9):
                                nc.tensor.matmul(
                                    lg[:], bones[s],
                                    prodmap[s][:, sub * 512 : sub * 512 + 512],
                                    start=(s == 0), stop=(s == 8),
                                )
                            nc.scalar.activation(
                                e_sb[:, ci * 512 : ci * 512 + 512], lg[:],
                                mybir.ActivationFunctionType.Exp,
                            )
                            sm = psB.tile([36, 512], F32, tag="sums", name=f"sm{ci}")
                            nc.tensor.matmul(
                                sm[:], sones2, e_sb[:, ci * 512 : ci * 512 + 512],
                                start=True, stop=True,
                            )
                            nc.scalar.activation(
                                t_sb[:, ci * 512 : ci * 512 + 512], sm[:],
                                mybir.ActivationFunctionType.Ln,
                            )
                            nc.scalar.activation(
                                r_exp[:, ci * 512 : ci * 512 + 512],
                                t_sb[:, ci * 512 : ci * 512 + 512],
                                mybir.ActivationFunctionType.Exp,
                                bias=lnalpha[:], scale=-1.0,
                            )

                    # m and c for this half, then round-trip + replication DMAs
                    nc.vector.tensor_mul(e_sb[:, sl], e_sb[:, sl], r_exp[:, sl])
                    if subtract_m:
                        nc.vector.tensor_sub(c_sb[:, sl], kern_sb[:, sl], e_sb[:, sl])
                    else:
                        nc.vector.tensor_add(c_sb[:, sl], kern_sb[:, sl], e_sb[:, sl])
                    nc.sync.dma_start(
                        bass.AP(c_ap.tensor, c_ap.offset + hf * 2048,
                                [[L, 36], [1, 2048]]),
                        c_sb[:, sl],
                    )

        # ---- Phase C: hybrid expansion + projection-accumulate -------------
        with tc.tile_pool(name="cexp", bufs=1) as pcex, \
             tc.tile_pool(name="psC", bufs=1, space="PSUM") as psC, \
             tc.tile_pool(name="psE", bufs=1, space="PSUM") as psE:
            cexp_dma = {}
            for hf in range(2):
                for s in DMA_SHIFTS:
                    if hf == 0:
                        cexp_dma[s] = pcex.tile([C, L], BF16, tag=f"cx{s}", name=f"cx{s}")
                    ce = cexp_dma[s]
                    for h in range(4):
                        src_ap = bass.AP(
                            c_ap.tensor,
                            c_ap.offset + (9 * h + s) * L + hf * 2048,
                            [[0, 32], [1, 2048]],
                        )
                        nc.sync.dma_start(
                            ce[32 * h : 32 * h + 32, hf * 2048 : hf * 2048 + 2048],
                            src_ap,
                        )

            out_sb = pcex.tile([C, L], BF16, name="out_sb")
            for hf in range(2):
                outps = [psC.tile([C, 512], F32, tag=f"out{cj}", name=f"outps{hf}_{cj}")
                         for cj in range(4)]
                for si, s in enumerate(S2_ORDER):
                    di, dj = SHIFTS[s]
                    if s in PE_SHIFTS:
                        pse = psE.tile([C, 2048], F32, tag="pse", name=f"pse{hf}_{s}")
                        for mc in range(4):
                            nc.tensor.matmul(
                                pse[:, mc * 512 : mc * 512 + 512],
                                eexp[s],
                                c_sb[:, hf * 2048 + mc * 512 : hf * 2048 + mc * 512 + 512],
                                start=True, stop=True, skip_group_check=True,
                            )
                        cesrc = pcex.tile([C, 2048], BF16, tag=f"ceh{si % 3}",
                                          name=f"ceh{hf}_{s}")
                        nc.scalar.copy(cesrc[:], pse[:])
                        ce_ap3 = cesrc[:].rearrange("p (r c) -> p r c", c=W)
                    else:
                        ce_ap3 = cexp_dma[s][:, hf * 2048 : hf * 2048 + 2048].rearrange(
                            "p (r c) -> p r c", c=W)
                    r0 = hf * 32
                    if dj == 0:
                        vsrc = vs3[:, 1 + di + r0 : 1 + di + r0 + 32, 0:W]
                    else:
                        vsrc = vp3[:, 1 + di + r0 : 1 + di + r0 + 32, 1 + dj : 1 + dj + W]
                    p2 = pcex.tile([C, 2048], BF16, tag=f"p2_{si % 3}", name=f"p2_{hf}_{s}")
                    nc.vector.tensor_mul(
                        p2[:].rearrange("p (r c) -> p r c", c=W), ce_ap3, vsrc,
                    )
                    for cj in range(4):
                        nc.tensor.matmul(
                            outps[cj][:],
                            wprojT,
                            p2[:, cj * 512 : cj * 512 + 512],
                            start=(si == 0), stop=(si == 8),
                            skip_group_check=True,
                        )
                for cj in range(4):
                    ci = hf * 4 + cj
                    nc.scalar.copy(out_sb[:, ci * 512 : ci * 512 + 512], outps[cj][:])
                    nc.sync.dma_start(
                        out_ext[:, ci * 512 : ci * 512 + 512],
                        out_sb[:, ci * 512 : ci * 512 + 512],
                    )

    nc.compile()
    return nc


_GRAPH_CACHE = {}


def _get_graph(subtract_m: bool):
    if subtract_m not in _GRAPH_CACHE:
        _GRAPH_CACHE[subtract_m] = _build(subtract_m)
    return _GRAPH_CACHE[subtract_m]


def prepare_feeds(x, w_qkv, w_kernel, b_kernel, w_proj, alpha, beta):
    x = np.asarray(x, np.float32)
    w_qkv = np.asarray(w_qkv, np.float32)
    w_kernel = np.asarray(w_kernel, np.float32)
    b_kernel = np.asarray(b_kernel, np.float32)
    w_proj = np.asarray(w_proj, np.float32)
    alpha = float(np.asarray(alpha))
    beta = float(np.asarray(beta))

    xb = x.reshape(B, C, L).astype(NPBF16)
    blob = np.zeros((C, WBLOB), np.float32)
    blob[:, OFF_QKV : OFF_QKV + 384] = w_qkv.T
    blob[:, OFF_PROJ : OFF_PROJ + 128] = w_proj.T
    wkb_full = np.zeros((36, 3 * C), np.float32)
    for g in range(4):
        wkb_full[g * 9 : g * 9 + 9, 96 * g : 96 * g + 96] = beta * w_kernel[g * 9 : g * 9 + 9]
    for j in range(3):
        blob[:, OFF_WKB + 36 * j : OFF_WKB + 36 * j + 36] = wkb_full[:, 128 * j : 128 * j + 128].T
    for h in range(4):
        blob[9 * h : 9 * h + 9, OFF_S2 + 9 * h : OFF_S2 + 9 * h + 9] = 1.0
    for s in range(9):
        for d in range(C):
            blob[d, OFF_BONES + 36 * s + 9 * (d // 32) + s] = 1.0
    for i, s in enumerate(PE_SHIFTS):
        for d in range(C):
            blob[9 * (d // 32) + s, OFF_EXP + 128 * i + d] = 1.0
    blob = blob.astype(NPBF16)
    bias = (beta * b_kernel).reshape(36, 1).astype(np.float32)
    ln_alpha = float(np.log(abs(alpha))) if alpha != 0.0 else -60.0
    lnalpha = np.full((36, 1), ln_alpha, np.float32)
    feeds = [
        {"x": np.ascontiguousarray(xb[b]), "wblob": blob, "bias": bias, "lnalpha": lnalpha}
        for b in range(B)
    ]
    return feeds, (alpha < 0.0)


def kernel(x, w_qkv, w_kernel, b_kernel, w_proj, alpha, beta):
    global LAST_RESULTS
    in_maps, subtract_m = prepare_feeds(x, w_qkv, w_kernel, b_kernel, w_proj, alpha, beta)
    nc = _get_graph(subtract_m)
    if TRACE:
        _ensure_profile_hook()
    res = run_bass_kernel_spmd(nc, in_maps, list(range(B)), trace=TRACE)
    LAST_RESULTS = res
    out = np.stack([np.asarray(res.results[b]["out"], np.float32) for b in range(B)])
    return out.reshape(B, C, H, W)


# revision 19
# speedup vs baseline: 1.4167x; 1.4167x over previous
"""ACmix (local 3x3 window attention + dynamic conv mix) on 8 TRN2 NeuronCores.

Sharding: data-parallel over batch B=8, one batch element per core.
Per-core layout: channels (128) on partitions, L = H*W = 4096 on the free dim.

Math per core (x: [128, L]):
  qkv   = w_qkv @ x                       (PE, 3 matmuls)
  q,k,v = split(qkv)
  k_pad, v_pad: zero-padded [128, 66*66] spatial buffers; a 3x3 shift is an AP offset
  logits[9h+s, l] = sum_{d in head h} q[d,l] * k_pad[d, l+off_s]
      -> DVE products q * k_shift (bf16, 2x), PE block-ones reduce into PSUM
  e = exp(logits); sums replicated per row via [36,36] block-ones matmul;
  r_exp = exp(-ln(sums)+ln|alpha|)  ([36, L], no round-trip needed)
  c[9h+s, l] = (beta*kernel + beta*b) +/- e*r_exp   (kernel = blockdiag(w_kernel)@qkv)
  c broadcast to 128 partitions via DRAM round-trip replication DMA
  out = w_proj @ sum_s c_exp_s * v_pad[:, l+off_s]   (PE accumulation, proj folded in)
"""

import os
from contextlib import ExitStack

import numpy as np
import ml_dtypes

import concourse.bass as bass
import concourse.bacc as bacc
import concourse.tile as tile
from concourse import mybir
from concourse.bass_utils import run_bass_kernel_spmd

BF16 = mybir.dt.bfloat16
F32 = mybir.dt.float32
NPBF16 = ml_dtypes.bfloat16

B, C, H, W = 8, 128, 64, 64
L = H * W                      # 4096
NH, HD, K2 = 4, 32, 9
PW, PH = W + 2, H + 2          # 66
PL = PW * PH                   # 4356
SHIFTS = [(di, dj) for di in (-1, 0, 1) for dj in (-1, 0, 1)]  # k2 = 3(di+1)+(dj+1)
NCHUNK = L // 512              # 8
NQ = L // 1024                 # 4 quarters for stage-1 products

# weight blob column layout (bf16, 128 partitions)
OFF_QKV = 0            # wqkvT [128, 384]
OFF_PROJ = 384         # wprojT [128, 128]
OFF_WKB = 512          # wkbT 3 x [128, 36]
OFF_S2 = 620           # sones2 [36, 36] (rows 36.. zero)
OFF_BONES = 656        # bones 9 x [128, 36]
OFF_EXP = 656 + 9 * 36  # 980: E_s [36 rows, 128] x PE-shifts
PE_SHIFTS = [3, 5, 8]
DMA_SHIFTS = [0, 1, 2, 4, 6, 7]
S2_ORDER = [0, 3, 1, 5, 2, 8, 4, 6, 7]
WBLOB = 980 + len(PE_SHIFTS) * 128

TRACE = False
LAST_RESULTS = None


def _ensure_profile_hook():
    """Provide antenv.axon_hooks (missing in this container's antenv stub)
    so run_bass_kernel_spmd(trace=True) can capture NTFF profiles."""
    import sys, types
    try:
        from antenv.axon_hooks import get_axon_ntff_profile_hook  # noqa: F401
        return
    except ImportError:
        pass
    try:
        from trn_agent_boot.trn_boot import _ntff_profile_via_ctypes
        hook = _ntff_profile_via_ctypes("/opt/axon/libaxon_pjrt.so")
    except Exception:
        hook = None
    mod = types.ModuleType("antenv.axon_hooks")
    state = {"hook": hook}
    mod.get_axon_ntff_profile_hook = lambda: state["hook"]
    mod.set_axon_ntff_profile_hook = lambda h: state.__setitem__("hook", h)
    sys.modules["antenv.axon_hooks"] = mod
    import antenv
    antenv.axon_hooks = mod


def _build(subtract_m: bool):
    nc = bacc.Bacc("TRN2", target_bir_lowering=False, debug=False)
    x_ext = nc.declare_dram_parameter("x", [C, L], BF16, isOutput=False)
    wblob_ext = nc.declare_dram_parameter("wblob", [C, WBLOB], BF16, isOutput=False)
    bias_ext = nc.declare_dram_parameter("bias", [36, 1], F32, isOutput=False)
    lnalpha_ext = nc.declare_dram_parameter("lnalpha", [36, 1], F32, isOutput=False)
    out_ext = nc.declare_dram_parameter("out", [C, L], BF16, isOutput=True)

    with tile.TileContext(nc) as tc, ExitStack() as ctx:
        pw = ctx.enter_context(tc.tile_pool(name="weights", bufs=1))
        pmain = ctx.enter_context(tc.tile_pool(name="main", bufs=1))
        pdram = ctx.enter_context(tc.tile_pool(name="dram", bufs=1, space="DRAM"))
        c_dram = pdram.tile([36, L], BF16, name="c_rt")

        # ---- S0: weights + input DMA, pad-border memsets -------------------
        wblob = pw.tile([C, WBLOB], BF16)
        nc.sync.dma_start(wblob[:], wblob_ext[:])
        bias_sb = pw.tile([36, 1], F32)
        nc.sync.dma_start(bias_sb[:], bias_ext[:])
        lnalpha = pw.tile([36, 1], F32)
        nc.sync.dma_start(lnalpha[:], lnalpha_ext[:])

        wqkvT = wblob[:, OFF_QKV : OFF_QKV + 384]
        wprojT = wblob[:, OFF_PROJ : OFF_PROJ + 128]
        wkbT = [wblob[:, OFF_WKB + 36 * j : OFF_WKB + 36 * j + 36] for j in range(3)]
        sones2 = wblob[0:36, OFF_S2 : OFF_S2 + 36]
        bones = [wblob[:, OFF_BONES + 36 * s : OFF_BONES + 36 * s + 36] for s in range(9)]

        k_pad = pmain.tile([C, PL], BF16)
        v_pad = pmain.tile([C, PL], BF16)
        k_stag = pmain.tile([C, PL], BF16)
        v_stag = pmain.tile([C, PL], BF16)
        for t in (k_pad, v_pad):
            t3 = t[:].rearrange("p (r c) -> p r c", c=PW)
            nc.gpsimd.memset(t3[:, 0, :], 0.0)          # top padded row
            nc.gpsimd.memset(t3[:, PH - 1, :], 0.0)     # bottom padded row
            nc.gpsimd.memset(t3[:, 1 : PH - 1, 0:1], 0.0)
            nc.gpsimd.memset(t3[:, 1 : PH - 1, PW - 1 : PW], 0.0)

        with tc.tile_pool(name="early", bufs=1) as pearly, \
             tc.tile_pool(name="prods", bufs=2) as pprod:
            x_sb = pearly.tile([C, L], BF16)
            for xq in range(4):
                nc.sync.dma_start(x_sb[:, xq * 1024 : xq * 1024 + 1024],
                                  x_ext[:, xq * 1024 : xq * 1024 + 1024])
            q_sb = pearly.tile([C, L], BF16)
            kern_sb = pmain.tile([36, L], BF16)
            e_sb = pmain.tile([36, L], BF16)
            t_sb = pmain.tile([36, L], F32)
            r_exp = pmain.tile([36, L], BF16)
            c_sb = pmain.tile([36, L], BF16)
            c_ap = c_dram[:]

            kp3 = k_pad[:].rearrange("p (r c) -> p r c", c=PW)
            vp3 = v_pad[:].rearrange("p (r c) -> p r c", c=PW)
            ks3 = k_stag[:].rearrange("p (r c) -> p r c", c=PW)
            vs3 = v_stag[:].rearrange("p (r c) -> p r c", c=PW)

            # ---- Phase A: qkv = w_qkv @ x (k first so products start early)
            with tc.tile_pool(name="psQ", bufs=2, space="PSUM") as psQ:
                for t in (1, 0, 2):  # k, q, v
                    for hf in range(2):
                        ps = psQ.tile([C, 2048], F32, tag="qkv")
                        for mc in range(4):
                            col = hf * 2048 + mc * 512
                            nc.tensor.matmul(
                                ps[:, mc * 512 : mc * 512 + 512],
                                wqkvT[:, t * C : t * C + C],
                                x_sb[:, col : col + 512],
                                start=True, stop=True,
                            )
                        if t == 0:
                            nc.scalar.copy(q_sb[:, hf * 2048 : hf * 2048 + 2048], ps[:])
                        else:
                            dst = (k_pad if t == 1 else v_pad)[:].rearrange(
                                "p (r c) -> p r c", c=PW
                            )
                            r0 = hf * 32
                            nc.scalar.copy(
                                dst[:, 1 + r0 : 1 + r0 + 32, 1 : 1 + W],
                                ps[:].rearrange("p (r c) -> p r c", c=W),
                            )
                    if t == 1:
                        nc.vector.tensor_copy(k_stag[:, 0 : PL - 2], k_pad[:, 1 : PL - 1])
                    elif t == 2:
                        nc.vector.tensor_copy(v_stag[:, 0 : PL - 2], v_pad[:, 1 : PL - 1])

            # ---- Phase B: fused per-half pipeline --------------------------
            eexp = {s: wblob[0:36, OFF_EXP + 128 * i : OFF_EXP + 128 * i + 128]
                    for i, s in enumerate(PE_SHIFTS)}
            with tc.tile_pool(name="psK", bufs=1, space="PSUM") as psK, \
                 tc.tile_pool(name="psB", bufs=2, space="PSUM") as psB:
                for hf in range(2):
                    sl = slice(hf * 2048, hf * 2048 + 2048)
                    # kernel proj for this half
                    psk = psK.tile([36, 2048], F32, tag="kern", name=f"kern{hf}")
                    r0 = hf * 32
                    rhs_k = kp3[:, 1 + r0 : 1 + r0 + 32, 1 : 1 + W]
                    rhs_v = vp3[:, 1 + r0 : 1 + r0 + 32, 1 : 1 + W]
                    for mc in range(4):
                        col = hf * 2048 + mc * 512
                        rr = mc * 8
                        nc.tensor.matmul(psk[:, mc * 512 : mc * 512 + 512], wkbT[0],
                                         q_sb[:, col : col + 512], start=True, stop=False)
                        nc.tensor.matmul(psk[:, mc * 512 : mc * 512 + 512], wkbT[1],
                                         rhs_k[:, rr : rr + 8, :], start=False, stop=False)
                        nc.tensor.matmul(psk[:, mc * 512 : mc * 512 + 512], wkbT[2],
                                         rhs_v[:, rr : rr + 8, :], start=False, stop=True)
                    nc.scalar.activation(
                        kern_sb[:, sl], psk[:],
                        mybir.ActivationFunctionType.Identity, bias=bias_sb[:],
                    )

                    for qq in range(2):
                        qt = hf * 2 + qq
                        prods = []
                        # dj != 0 shifts first: they don't wait on the stag copy
                        order = [s for s in range(9) if SHIFTS[s][1] != 0] + \
                                [s for s in range(9) if SHIFTS[s][1] == 0]
                        prodmap = {}
                        for s in order:
                            di, dj = SHIFTS[s]
                            pr = pprod.tile([C, 1024], BF16, tag=f"pr{s}", name=f"pr{s}_{qt}")
                            r0q = qt * 16
                            if dj == 0:
                                psrc = ks3[:, 1 + di + r0q : 1 + di + r0q + 16, 0:W]
                            else:
                                psrc = kp3[:, 1 + di + r0q : 1 + di + r0q + 16,
                                           1 + dj : 1 + dj + W]
                            nc.vector.tensor_mul(
                                pr[:].rearrange("p (r c) -> p r c", c=W),
                                q_sb[:, qt * 1024 : qt * 1024 + 1024].rearrange(
                                    "p (r c) -> p r c", c=W),
                                psrc,
                            )
                            prodmap[s] = pr
                        for sub in range(2):
                            ci = qt * 2 + sub
                            lg = psB.tile([36, 512], F32, tag="logits", name=f"lg{ci}")
                            for s in range(9):
                                nc.tensor.matmul(
                                    lg[:], bones[s],
                                    prodmap[s][:, sub * 512 : sub * 512 + 512],
                                    start=(s == 0), stop=(s == 8),
                                )
                            nc.scalar.activation(
                                e_sb[:, ci * 512 : ci * 512 + 512], lg[:],
                                mybir.ActivationFunctionType.Exp,
                            )
                            sm = psB.tile([36, 512], F32, tag="sums", name=f"sm{ci}")
                            nc.tensor.matmul(
                                sm[:], sones2, e_sb[:, ci * 512 : ci * 512 + 512],
                                start=True, stop=True,
                            )
                            nc.scalar.activation(
                                t_sb[:, ci * 512 : ci * 512 + 512], sm[:],
                                mybir.ActivationFunctionType.Ln,
                            )
                            nc.scalar.activation(
                                r_exp[:, ci * 512 : ci * 512 + 512],
                                t_sb[:, ci * 512 : ci * 512 + 512],
                                mybir.ActivationFunctionType.Exp,
                                bias=lnalpha[:], scale=-1.0,
                            )

                    # m and c for this half, then round-trip + replication DMAs
                    nc.vector.tensor_mul(e_sb[:, sl], e_sb[:, sl], r_exp[:, sl])
                    if subtract_m:
                        nc.vector.tensor_sub(c_sb[:, sl], kern_sb[:, sl], e_sb[:, sl])
                    else:
                        nc.vector.tensor_add(c_sb[:, sl], kern_sb[:, sl], e_sb[:, sl])
                    nc.sync.dma_start(
                        bass.AP(c_ap.tensor, c_ap.offset + hf * 2048,
                                [[L, 36], [1, 2048]]),
                        c_sb[:, sl],
                    )

        # ---- Phase C: hybrid expansion + projection-accumulate -------------
        with tc.tile_pool(name="cexp", bufs=1) as pcex, \
             tc.tile_pool(name="psC", bufs=1, space="PSUM") as psC, \
             tc.tile_pool(name="psE", bufs=1, space="PSUM") as psE:
            cexp_dma = {}
            for hf in range(2):
                for s in DMA_SHIFTS:
                    if hf == 0:
                        cexp_dma[s] = pcex.tile([C, L], BF16, tag=f"cx{s}", name=f"cx{s}")
                    ce = cexp_dma[s]
                    for h in range(4):
                        src_ap = bass.AP(
                            c_ap.tensor,
                            c_ap.offset + (9 * h + s) * L + hf * 2048,
                            [[0, 32], [1, 2048]],
                        )
                        nc.sync.dma_start(
                            ce[32 * h : 32 * h + 32, hf * 2048 : hf * 2048 + 2048],
                            src_ap,
                        )

            out_sb = pcex.tile([C, L], BF16, name="out_sb")
            for hf in range(2):
                outps = [psC.tile([C, 512], F32, tag=f"out{cj}", name=f"outps{hf}_{cj}")
                         for cj in range(4)]
                for si, s in enumerate(S2_ORDER):
                    di, dj = SHIFTS[s]
                    if s in PE_SHIFTS:
                        pse = psE.tile([C, 2048], F32, tag="pse", name=f"pse{hf}_{s}")
                        for mc in range(4):
                            nc.tensor.matmul(
                                pse[:, mc * 512 : mc * 512 + 512],
                                eexp[s],
                                c_sb[:, hf * 2048 + mc * 512 : hf * 2048 + mc * 512 + 512],
                                start=True, stop=True, skip_group_check=True,
                            )
                        cesrc = pcex.tile([C, 2048], BF16, tag=f"ceh{si % 3}",
                                          name=f"ceh{hf}_{s}")
                        nc.scalar.copy(cesrc[:], pse[:])
                        ce_ap3 = cesrc[:].rearrange("p (r c) -> p r c", c=W)
                    else:
                        ce_ap3 = cexp_dma[s][:, hf * 2048 : hf * 2048 + 2048].rearrange(
                            "p (r c) -> p r c", c=W)
                    r0 = hf * 32
                    if dj == 0:
                        vsrc = vs3[:, 1 + di + r0 : 1 + di + r0 + 32, 0:W]
                    else:
                        vsrc = vp3[:, 1 + di + r0 : 1 + di + r0 + 32, 1 + dj : 1 + dj + W]
                    p2 = pcex.tile([C, 2048], BF16, tag=f"p2_{si % 3}", name=f"p2_{hf}_{s}")
                    nc.vector.tensor_mul(
                        p2[:].rearrange("p (r c) -> p r c", c=W), ce_ap3, vsrc,
                    )
                    for cj in range(4):
                        nc.tensor.matmul(
                            outps[cj][:],
                            wprojT,
                            p2[:, cj * 512 : cj * 512 + 512],
                            start=(si == 0), stop=(si == 8),
                            skip_group_check=True,
                        )
                for cj in range(4):
                    ci = hf * 4 + cj
                    nc.scalar.copy(out_sb[:, ci * 512 : ci * 512 + 512], outps[cj][:])
                    nc.sync.dma_start(
                        out_ext[:, ci * 512 : ci * 512 + 512],
                        out_sb[:, ci * 512 : ci * 512 + 512],
                    )

    nc.compile()
    return nc


_GRAPH_CACHE = {}


def _get_graph(subtract_m: bool):
    if subtract_m not in _GRAPH_CACHE:
        _GRAPH_CACHE[subtract_m] = _build(subtract_m)
    return _GRAPH_CACHE[subtract_m]


def prepare_feeds(x, w_qkv, w_kernel, b_kernel, w_proj, alpha, beta):
    x = np.asarray(x, np.float32)
    w_qkv = np.asarray(w_qkv, np.float32)
    w_kernel = np.asarray(w_kernel, np.float32)
    b_kernel = np.asarray(b_kernel, np.float32)
    w_proj = np.asarray(w_proj, np.float32)
    alpha = float(np.asarray(alpha))
    beta = float(np.asarray(beta))

    xb = x.reshape(B, C, L).astype(NPBF16)
    blob = np.zeros((C, WBLOB), np.float32)
    blob[:, OFF_QKV : OFF_QKV + 384] = w_qkv.T
    blob[:, OFF_PROJ : OFF_PROJ + 128] = w_proj.T
    wkb_full = np.zeros((36, 3 * C), np.float32)
    for g in range(4):
        wkb_full[g * 9 : g * 9 + 9, 96 * g : 96 * g + 96] = beta * w_kernel[g * 9 : g * 9 + 9]
    for j in range(3):
        blob[:, OFF_WKB + 36 * j : OFF_WKB + 36 * j + 36] = wkb_full[:, 128 * j : 128 * j + 128].T
    for h in range(4):
        blob[9 * h : 9 * h + 9, OFF_S2 + 9 * h : OFF_S2 + 9 * h + 9] = 1.0
    for s in range(9):
        for d in range(C):
            blob[d, OFF_BONES + 36 * s + 9 * (d // 32) + s] = 1.0
    for i, s in enumerate(PE_SHIFTS):
        for d in range(C):
            blob[9 * (d // 32) + s, OFF_EXP + 128 * i + d] = 1.0
    blob = blob.astype(NPBF16)
    bias = (beta * b_kernel).reshape(36, 1).astype(np.float32)
    ln_alpha = float(np.log(abs(alpha))) if alpha != 0.0 else -60.0
    lnalpha = np.full((36, 1), ln_alpha, np.float32)
    feeds = [
        {"x": np.ascontiguousarray(xb[b]), "wblob": blob, "bias": bias, "lnalpha": lnalpha}
        for b in range(B)
    ]
    return feeds, (alpha < 0.0)


def kernel(x, w_qkv, w_kernel, b_kernel, w_proj, alpha, beta):
    global LAST_RESULTS
    in_maps, subtract_m = prepare_feeds(x, w_qkv, w_kernel, b_kernel, w_proj, alpha, beta)
    nc = _get_graph(subtract_m)
    if TRACE:
        _ensure_profile_hook()
    res = run_bass_kernel_spmd(nc, in_maps, list(range(B)), trace=TRACE)
    LAST_RESULTS = res
    out = np.stack([np.asarray(res.results[b]["out"], np.float32) for b in range(B)])
    return out.reshape(B, C, H, W)


# revision 21
# speedup vs baseline: 1.5911x; 1.1231x over previous
"""ACmix (local 3x3 window attention + dynamic conv mix) on 8 TRN2 NeuronCores.

Sharding: data-parallel over batch B=8, one batch element per core.
Per-core layout: channels (128) on partitions, L = H*W = 4096 on the free dim.

Math per core (x: [128, L]):
  qkv   = w_qkv @ x                       (PE, 3 matmuls)
  q,k,v = split(qkv)
  k_pad, v_pad: zero-padded [128, 66*66] spatial buffers; a 3x3 shift is an AP offset
  logits[9h+s, l] = sum_{d in head h} q[d,l] * k_pad[d, l+off_s]
      -> DVE products q * k_shift (bf16, 2x), PE block-ones reduce into PSUM
  e = exp(logits); sums replicated per row via [36,36] block-ones matmul;
  r_exp = exp(-ln(sums)+ln|alpha|)  ([36, L], no round-trip needed)
  c[9h+s, l] = (beta*kernel + beta*b) +/- e*r_exp   (kernel = blockdiag(w_kernel)@qkv)
  c broadcast to 128 partitions via DRAM round-trip replication DMA
  out = w_proj @ sum_s c_exp_s * v_pad[:, l+off_s]   (PE accumulation, proj folded in)
"""

import os
from contextlib import ExitStack

import numpy as np
import ml_dtypes

import concourse.bass as bass
import concourse.bacc as bacc
import concourse.tile as tile
from concourse import mybir
from concourse.bass_utils import run_bass_kernel_spmd

BF16 = mybir.dt.bfloat16
F32 = mybir.dt.float32
NPBF16 = ml_dtypes.bfloat16

B, C, H, W = 8, 128, 64, 64
L = H * W                      # 4096
NH, HD, K2 = 4, 32, 9
PW, PH = W + 2, H + 2          # 66
PL = PW * PH                   # 4356
SHIFTS = [(di, dj) for di in (-1, 0, 1) for dj in (-1, 0, 1)]  # k2 = 3(di+1)+(dj+1)
NCHUNK = L // 512              # 8
NQ = L // 1024                 # 4 quarters for stage-1 products

# weight blob column layout (bf16, 128 partitions)
OFF_QKV = 0            # wqkvT [128, 384]
OFF_PROJ = 384         # wprojT [128, 128]
OFF_WKB = 512          # wkbT 3 x [128, 36]
OFF_S2 = 620           # sones2 [36, 36] (rows 36.. zero)
OFF_BONES = 656        # bones 9 x [128, 36]
OFF_EXP = 656 + 9 * 36  # 980: E_s [36 rows, 128] x PE-shifts
PE_SHIFTS = [3, 5, 8]
DMA_SHIFTS = [0, 1, 2, 4, 6, 7]
S2_ORDER = [3, 5, 8, 0, 1, 2, 4, 6, 7]
WBLOB = 980 + len(PE_SHIFTS) * 128

TRACE = False
LAST_RESULTS = None


def _ensure_profile_hook():
    """Provide antenv.axon_hooks (missing in this container's antenv stub)
    so run_bass_kernel_spmd(trace=True) can capture NTFF profiles."""
    import sys, types
    try:
        from antenv.axon_hooks import get_axon_ntff_profile_hook  # noqa: F401
        return
    except ImportError:
        pass
    try:
        from trn_agent_boot.trn_boot import _ntff_profile_via_ctypes
        hook = _ntff_profile_via_ctypes("/opt/axon/libaxon_pjrt.so")
    except Exception:
        hook = None
    mod = types.ModuleType("antenv.axon_hooks")
    state = {"hook": hook}
    mod.get_axon_ntff_profile_hook = lambda: state["hook"]
    mod.set_axon_ntff_profile_hook = lambda h: state.__setitem__("hook", h)
    sys.modules["antenv.axon_hooks"] = mod
    import antenv
    antenv.axon_hooks = mod


def _build(subtract_m: bool):
    nc = bacc.Bacc("TRN2", target_bir_lowering=False, debug=False)
    x_ext = nc.declare_dram_parameter("x", [C, L], BF16, isOutput=False)
    wblob_ext = nc.declare_dram_parameter("wblob", [C, WBLOB], BF16, isOutput=False)
    bias_ext = nc.declare_dram_parameter("bias", [36, 1], F32, isOutput=False)
    out_ext = nc.declare_dram_parameter("out", [C, L], BF16, isOutput=True)

    with tile.TileContext(nc) as tc, ExitStack() as ctx:
        pw = ctx.enter_context(tc.tile_pool(name="weights", bufs=1))
        pmain = ctx.enter_context(tc.tile_pool(name="main", bufs=1))
        pdram = ctx.enter_context(tc.tile_pool(name="dram", bufs=1, space="DRAM"))
        c_dram = pdram.tile([36, L], BF16, name="c_rt")

        # ---- S0: weights + input DMA, pad-border memsets -------------------
        wblob = pw.tile([C, WBLOB], BF16)
        nc.sync.dma_start(wblob[:], wblob_ext[:])
        bias_sb = pw.tile([36, 1], F32)
        nc.sync.dma_start(bias_sb[:], bias_ext[:])

        wqkvT = wblob[:, OFF_QKV : OFF_QKV + 384]
        wprojT = wblob[:, OFF_PROJ : OFF_PROJ + 128]
        wkbT = [wblob[:, OFF_WKB + 36 * j : OFF_WKB + 36 * j + 36] for j in range(3)]
        sones2 = wblob[0:36, OFF_S2 : OFF_S2 + 36]
        bones = [wblob[:, OFF_BONES + 36 * s : OFF_BONES + 36 * s + 36] for s in range(9)]

        k_pad = pmain.tile([C, PL], BF16)
        v_pad = pmain.tile([C, PL], BF16)
        k_stag = pmain.tile([C, PL], BF16)
        v_stag = pmain.tile([C, PL], BF16)
        for t in (k_pad, v_pad):
            t3 = t[:].rearrange("p (r c) -> p r c", c=PW)
            nc.gpsimd.memset(t3[:, 0, :], 0.0)          # top padded row
            nc.gpsimd.memset(t3[:, PH - 1, :], 0.0)     # bottom padded row
            nc.gpsimd.memset(t3[:, 1 : PH - 1, 0:1], 0.0)
            nc.gpsimd.memset(t3[:, 1 : PH - 1, PW - 1 : PW], 0.0)

        with tc.tile_pool(name="early", bufs=1) as pearly, \
             tc.tile_pool(name="prods", bufs=2) as pprod:
            x_sb = pearly.tile([C, L], BF16)
            for xq in range(4):
                nc.sync.dma_start(x_sb[:, xq * 1024 : xq * 1024 + 1024],
                                  x_ext[:, xq * 1024 : xq * 1024 + 1024])
            q_sb = pearly.tile([C, L], BF16)
            kern_sb = pmain.tile([36, L], BF16)
            e_sb = pmain.tile([36, L], BF16)
            r_exp = pmain.tile([36, L], F32)
            c_sb = pmain.tile([36, L], BF16)
            c_ap = c_dram[:]

            kp3 = k_pad[:].rearrange("p (r c) -> p r c", c=PW)
            vp3 = v_pad[:].rearrange("p (r c) -> p r c", c=PW)
            ks3 = k_stag[:].rearrange("p (r c) -> p r c", c=PW)
            vs3 = v_stag[:].rearrange("p (r c) -> p r c", c=PW)

            # ---- Phase A: qkv = w_qkv @ x (k first so products start early)
            with tc.tile_pool(name="psQ", bufs=2, space="PSUM") as psQ:
                for t in (1, 0, 2):  # k, q, v
                    for hf in range(2):
                        ps = psQ.tile([C, 2048], F32, tag="qkv")
                        for mc in range(4):
                            col = hf * 2048 + mc * 512
                            nc.tensor.matmul(
                                ps[:, mc * 512 : mc * 512 + 512],
                                wqkvT[:, t * C : t * C + C],
                                x_sb[:, col : col + 512],
                                start=True, stop=True,
                            )
                        if t == 0:
                            nc.scalar.copy(q_sb[:, hf * 2048 : hf * 2048 + 2048], ps[:])
                        else:
                            dst = (k_pad if t == 1 else v_pad)[:].rearrange(
                                "p (r c) -> p r c", c=PW
                            )
                            r0 = hf * 32
                            nc.scalar.copy(
                                dst[:, 1 + r0 : 1 + r0 + 32, 1 : 1 + W],
                                ps[:].rearrange("p (r c) -> p r c", c=W),
                            )
                    if t == 1:
                        nc.vector.tensor_copy(k_stag[:, 0 : PL - 2], k_pad[:, 1 : PL - 1])
                    elif t == 2:
                        nc.vector.tensor_copy(v_stag[:, 0 : PL - 2], v_pad[:, 1 : PL - 1])

            # ---- Phase B: fused per-half pipeline --------------------------
            eexp = {s: wblob[0:36, OFF_EXP + 128 * i : OFF_EXP + 128 * i + 128]
                    for i, s in enumerate(PE_SHIFTS)}
            with tc.tile_pool(name="psK", bufs=1, space="PSUM") as psK, \
                 tc.tile_pool(name="psB", bufs=2, space="PSUM") as psB:
                for hf in range(2):
                    sl = slice(hf * 2048, hf * 2048 + 2048)
                    # kernel proj for this half
                    psk = psK.tile([36, 2048], F32, tag="kern", name=f"kern{hf}")
                    r0 = hf * 32
                    rhs_k = kp3[:, 1 + r0 : 1 + r0 + 32, 1 : 1 + W]
                    rhs_v = vp3[:, 1 + r0 : 1 + r0 + 32, 1 : 1 + W]
                    for mc in range(4):
                        col = hf * 2048 + mc * 512
                        rr = mc * 8
                        nc.tensor.matmul(psk[:, mc * 512 : mc * 512 + 512], wkbT[0],
                                         q_sb[:, col : col + 512], start=True, stop=False)
                        nc.tensor.matmul(psk[:, mc * 512 : mc * 512 + 512], wkbT[1],
                                         rhs_k[:, rr : rr + 8, :], start=False, stop=False)
                        nc.tensor.matmul(psk[:, mc * 512 : mc * 512 + 512], wkbT[2],
                                         rhs_v[:, rr : rr + 8, :], start=False, stop=True)
                    nc.scalar.activation(
                        kern_sb[:, sl], psk[:],
                        mybir.ActivationFunctionType.Identity, bias=bias_sb[:],
                    )

                    for qq in range(2):
                        qt = hf * 2 + qq
                        prods = []
                        # dj != 0 shifts first: they don't wait on the stag copy
                        order = [s for s in range(9) if SHIFTS[s][1] != 0] + \
                                [s for s in range(9) if SHIFTS[s][1] == 0]
                        prodmap = {}
                        for s in order:
                            di, dj = SHIFTS[s]
                            pr = pprod.tile([C, 1024], BF16, tag=f"pr{s}", name=f"pr{s}_{qt}")
                            r0q = qt * 16
                            if dj == 0:
                                psrc = ks3[:, 1 + di + r0q : 1 + di + r0q + 16, 0:W]
                            else:
                                psrc = kp3[:, 1 + di + r0q : 1 + di + r0q + 16,
                                           1 + dj : 1 + dj + W]
                            nc.vector.tensor_mul(
                                pr[:].rearrange("p (r c) -> p r c", c=W),
                                q_sb[:, qt * 1024 : qt * 1024 + 1024].rearrange(
                                    "p (r c) -> p r c", c=W),
                                psrc,
                            )
                            prodmap[s] = pr
                        for sub in range(2):
                            ci = qt * 2 + sub
                            lg = psB.tile([36, 512], F32, tag="logits", name=f"lg{ci}")
                            for s in range(9):
                                nc.tensor.matmul(
                                    lg[:], bones[s],
                                    prodmap[s][:, sub * 512 : sub * 512 + 512],
                                    start=(s == 0), stop=(s == 8),
                                )
                            nc.scalar.activation(
                                e_sb[:, ci * 512 : ci * 512 + 512], lg[:],
                                mybir.ActivationFunctionType.Exp,
                            )
                            if subtract_m:  # alpha == 0: attn branch off
                                nc.vector.memset(
                                    r_exp[:, ci * 512 : ci * 512 + 512], 0.0)
                            else:
                                sm = psB.tile([36, 512], F32, tag="sums", name=f"sm{ci}")
                                nc.tensor.matmul(
                                    sm[:], sones2, e_sb[:, ci * 512 : ci * 512 + 512],
                                    start=True, stop=True,
                                )
                                nc.vector.reciprocal_approx_fast(
                                    r_exp[:, ci * 512 : ci * 512 + 512], sm[:])

                    # m and c for this half, then round-trip + replication DMAs
                    nc.vector.tensor_mul(e_sb[:, sl], e_sb[:, sl], r_exp[:, sl])
                    nc.vector.tensor_add(c_sb[:, sl], kern_sb[:, sl], e_sb[:, sl])
                    nc.sync.dma_start(
                        bass.AP(c_ap.tensor, c_ap.offset + hf * 2048,
                                [[L, 36], [1, 2048]]),
                        c_sb[:, sl],
                    )

        # ---- Phase C: hybrid expansion + projection-accumulate -------------
        with tc.tile_pool(name="cexp", bufs=1) as pcex, \
             tc.tile_pool(name="psC", bufs=1, space="PSUM") as psC, \
             tc.tile_pool(name="psE", bufs=2, space="PSUM") as psE:
            cexp_dma = {}
            for hf in range(2):
                for s in DMA_SHIFTS:
                    if hf == 0:
                        cexp_dma[s] = pcex.tile([C, L], BF16, tag=f"cx{s}", name=f"cx{s}")
                    ce = cexp_dma[s]
                    for h in range(4):
                        src_ap = bass.AP(
                            c_ap.tensor,
                            c_ap.offset + (9 * h + s) * L + hf * 2048,
                            [[0, 32], [1, 2048]],
                        )
                        nc.sync.dma_start(
                            ce[32 * h : 32 * h + 32, hf * 2048 : hf * 2048 + 2048],
                            src_ap,
                        )

            out_sb = pcex.tile([C, L], BF16, name="out_sb")
            for hf in range(2):
                outps = [psC.tile([C, 512], F32, tag=f"out{cj}", name=f"outps{hf}_{cj}")
                         for cj in range(4)]
                for si, s in enumerate(S2_ORDER):
                    di, dj = SHIFTS[s]
                    if s in PE_SHIFTS:
                        cesrc = pcex.tile([C, 2048], BF16, tag=f"ceh{si % 3}",
                                          name=f"ceh{hf}_{s}")
                        for qq in range(2):
                            pse = psE.tile([C, 1024], F32, tag="pse",
                                           name=f"pse{hf}_{s}_{qq}")
                            for mc in range(2):
                                col = qq * 1024 + mc * 512
                                nc.tensor.matmul(
                                    pse[:, mc * 512 : mc * 512 + 512],
                                    eexp[s],
                                    c_sb[:, hf * 2048 + col : hf * 2048 + col + 512],
                                    start=True, stop=True, skip_group_check=True,
                                )
                            nc.scalar.copy(
                                cesrc[:, qq * 1024 : qq * 1024 + 1024], pse[:])
                        ce_ap3 = cesrc[:].rearrange("p (r c) -> p r c", c=W)
                    else:
                        ce_ap3 = cexp_dma[s][:, hf * 2048 : hf * 2048 + 2048].rearrange(
                            "p (r c) -> p r c", c=W)
                    r0 = hf * 32
                    if dj == 0:
                        vsrc = vs3[:, 1 + di + r0 : 1 + di + r0 + 32, 0:W]
                    else:
                        vsrc = vp3[:, 1 + di + r0 : 1 + di + r0 + 32, 1 + dj : 1 + dj + W]
                    p2 = pcex.tile([C, 2048], BF16, tag=f"p2_{si % 3}", name=f"p2_{hf}_{s}")
                    nc.vector.tensor_mul(
                        p2[:].rearrange("p (r c) -> p r c", c=W), ce_ap3, vsrc,
                    )
                    for cj in range(4):
                        nc.tensor.matmul(
                            outps[cj][:],
                            wprojT,
                            p2[:, cj * 512 : cj * 512 + 512],
                            start=(si == 0), stop=(si == 8),
                            skip_group_check=True,
                        )
                for cj in range(4):
                    ci = hf * 4 + cj
                    nc.scalar.copy(out_sb[:, ci * 512 : ci * 512 + 512], outps[cj][:])
                    nc.sync.dma_start(
                        out_ext[:, ci * 512 : ci * 512 + 512],
                        out_sb[:, ci * 512 : ci * 512 + 512],
                    )

    nc.compile()
    return nc


_GRAPH_CACHE = {}


def _get_graph(subtract_m: bool):
    if subtract_m not in _GRAPH_CACHE:
        _GRAPH_CACHE[subtract_m] = _build(subtract_m)
    return _GRAPH_CACHE[subtract_m]


def prepare_feeds(x, w_qkv, w_kernel, b_kernel, w_proj, alpha, beta):
    x = np.asarray(x, np.float32)
    w_qkv = np.asarray(w_qkv, np.float32)
    w_kernel = np.asarray(w_kernel, np.float32)
    b_kernel = np.asarray(b_kernel, np.float32)
    w_proj = np.asarray(w_proj, np.float32)
    alpha = float(np.asarray(alpha))
    beta = float(np.asarray(beta))

    # Fold alpha into the output projection and beta/alpha into the kernel
    # branch so the attention coefficient is exactly e/sums on device.
    alpha0 = (alpha == 0.0)
    if alpha0:
        proj_scale, kb_scale = 1.0, beta
    else:
        proj_scale, kb_scale = alpha, beta / alpha

    xb = x.reshape(B, C, L).astype(NPBF16)
    blob = np.zeros((C, WBLOB), np.float32)
    blob[:, OFF_QKV : OFF_QKV + 384] = w_qkv.T
    blob[:, OFF_PROJ : OFF_PROJ + 128] = proj_scale * w_proj.T
    wkb_full = np.zeros((36, 3 * C), np.float32)
    for g in range(4):
        wkb_full[g * 9 : g * 9 + 9, 96 * g : 96 * g + 96] = kb_scale * w_kernel[g * 9 : g * 9 + 9]
    for j in range(3):
        blob[:, OFF_WKB + 36 * j : OFF_WKB + 36 * j + 36] = wkb_full[:, 128 * j : 128 * j + 128].T
    for h in range(4):
        blob[9 * h : 9 * h + 9, OFF_S2 + 9 * h : OFF_S2 + 9 * h + 9] = 1.0
    for s in range(9):
        for d in range(C):
            blob[d, OFF_BONES + 36 * s + 9 * (d // 32) + s] = 1.0
    for i, s in enumerate(PE_SHIFTS):
        for d in range(C):
            blob[9 * (d // 32) + s, OFF_EXP + 128 * i + d] = 1.0
    blob = blob.astype(NPBF16)
    bias = (kb_scale * b_kernel).reshape(36, 1).astype(np.float32)
    feeds = [
        {"x": np.ascontiguousarray(xb[b]), "wblob": blob, "bias": bias}
        for b in range(B)
    ]
    return feeds, alpha0


def kernel(x, w_qkv, w_kernel, b_kernel, w_proj, alpha, beta):
    global LAST_RESULTS
    in_maps, subtract_m = prepare_feeds(x, w_qkv, w_kernel, b_kernel, w_proj, alpha, beta)
    nc = _get_graph(subtract_m)
    if TRACE:
        _ensure_profile_hook()
    res = run_bass_kernel_spmd(nc, in_maps, list(range(B)), trace=TRACE)
    LAST_RESULTS = res
    out = np.stack([np.asarray(res.results[b]["out"], np.float32) for b in range(B)])
    return out.reshape(B, C, H, W)
